# revision 28
# baseline (speedup 1.0000x reference)
"""Trainium2 Bass kernel for batched 7-DOF RNEA inverse dynamics.

Layout: pure data-parallel over 8 NeuronCores (32768 batch elements each).
Per core, every per-element scalar lives as an SBUF "plane" [128, 256] f32
(batch element e = partition*256 + free). All per-link parameters (trans,
rot_fix, mass, com, inertia, damping) are baked into the instruction stream
as immediates at build time.

The math is emitted through a small expression compiler:
  Expr = sum(coef * plane) + const
Linear combinations are free (term concatenation); they materialize as
chains of fused scalar_tensor_tensor ops ((in0*imm) + in1, in-place
accumulation) only when a product or an output needs a raw plane. Products
are DVE tensor_tensor ops. sin/cos and the final affine outputs go to the
scalar engine (ACT).

Algebraic structure used:
  - Rinv @ x with Rinv = Rz(q)^T @ rot_fix^T: constant matvec folded into
    stt-chain immediates, then one complex rotation (4 products).
  - cross(t_inv, Rinv x) = -Rinv (t x x): constant cross matrices fold into
    linear immediates, killing 12 products per forward joint.
  - forward state for joint 6 is never computed (the reference's backward
    recursion never reads it).
  - ACT Sin has no range reduction (accurate only on [-pi, pi]) and
    q ~ N(0,1) exceeds pi; half-angle identities avoid wrapping:
    cos q = 1-2sin^2(q/2), sin q = 2 sin(q/2)(1-2sin^2(q/4)).

Measured (8 cores, trn2): HW exec ~368 us, max abs err ~1.1e-4 on torque
absmax ~165 (fp32). Vector engine is the bottleneck at ~95% occupancy
(~1000 DVE ops x ~366 ns); TensorE was evaluated and rejected (fp32 matmul
4 cyc/row, fp32r ~677 ns/matmul + per-stage weight reloads + 1.6e-4
rounding per pass), GPSIMD rejected (no scalar_tensor_tensor opcode on
Pool), bf16 rejected (2e-2 absmax error for only 10% speedup).
"""

import os
import sys

for _p in ("/opt/trn_rl_repo",):
    if os.path.isdir(_p) and _p not in sys.path:
        sys.path.append(_p)

import numpy as np

import concourse.bass as bass
import concourse.bacc as bacc
import concourse.mybir as mybir
from concourse import tile as tile_mod
from concourse import bass_utils

N_CORES = 8
ND = 7
B_TOTAL = 262144
BC = B_TOTAL // N_CORES  # 32768 per core
P = 128
F = BC // P  # 256
GRAV = 9.81
DT = mybir.dt.float32
DT_C = mybir.dt.bfloat16 if os.environ.get("K_BF16", "0") == "1" else mybir.dt.float32
OP = mybir.AluOpType
AF = mybir.ActivationFunctionType


class Expr:
    """value = sum(coef * plane_ap) + const"""

    __slots__ = ("terms", "const", "_mat")

    def __init__(self, terms=(), const=0.0):
        self.terms = list(terms)
        self.const = float(const)
        self._mat = None  # cached (coef, ap) of materialized sum-of-terms


ZERO = Expr()


def _nonzero(e):
    return bool(e.terms) or e.const != 0.0


class Builder:
    def __init__(self, nc, pool, ring_size=64, pool_frac=0.0):
        self.nc = nc
        self.pool = pool
        self.n_stt = 0
        self.n_tt = 0
        self.n_act = 0
        self.n_copy = 0
        # engine load balancing between DVE and GPSIMD (2-input ops)
        self.eng_busy = [0.0, 0.0]  # ns accumulated: [vector, gpsimd]
        self.eng_cost = [445.0, 980.0]
        self.pool_frac = pool_frac
        self.ring_size = ring_size
        self.ring_idx = 0
        self.joint_allocs = 0
        self.max_joint_allocs = 0
        self.pers_idx = 0
        self.free_tags = []       # recycled persistent tags
        self.free_wide = {}       # recycled wide tags by slot count
        self.pers_ids = set()     # id(ap) of planes safe to reference long-term
        self.ap_tag = {}          # id(ap) -> tag (for freeing)

    def pick_engine(self, n_ops=1):
        """Pick vector or gpsimd for a chain of n_ops 2-input ops."""
        if self.pool_frac <= 0.0:
            self.eng_busy[0] += n_ops * self.eng_cost[0]
            return self.nc.vector
        c0 = self.eng_busy[0] + n_ops * self.eng_cost[0]
        c1 = self.eng_busy[1] + n_ops * self.eng_cost[1]
        if c1 < c0:
            self.eng_busy[1] = c1
            return self.nc.gpsimd
        self.eng_busy[0] = c0
        return self.nc.vector

    # ---- allocation ----
    def scratch(self, dtype=None):
        tag = f"s{self.ring_idx % self.ring_size}"
        t = self.pool.tile([P, F], dtype or DT_C, tag=tag, name=tag)
        self.ring_idx += 1
        self.joint_allocs += 1
        return t[:, :]

    def persistent(self, label=""):
        if self.free_tags:
            tag = self.free_tags.pop()
        else:
            tag = f"p{self.pers_idx}"
            self.pers_idx += 1
        t = self.pool.tile([P, F], DT_C, tag=tag, name=f"{tag}_{label}")
        ap = t[:, :]
        self.pers_ids.add(id(ap))
        self.ap_tag[id(ap)] = ("p", tag)
        return ap

    def wscratch(self, slots):
        """Wide scratch tile [P, slots*F] from a per-width ring."""
        if not hasattr(self, "wring"):
            self.wring = {}
        idx = self.wring.get(slots, 0)
        self.wring[slots] = idx + 1
        tag = f"w{slots}_{idx % 6}"
        t = self.pool.tile([P, slots * F], DT_C, tag=tag, name=tag)
        return t[:, :]

    def persistent_wide(self, slots, label=""):
        if not hasattr(self, "pwide_idx"):
            self.pwide_idx = 0
        fl = self.free_wide.setdefault(slots, [])
        if fl:
            tag = fl.pop()
        else:
            tag = f"pw{slots}_{self.pwide_idx}"
            self.pwide_idx += 1
        t = self.pool.tile([P, slots * F], DT_C, tag=tag, name=f"{tag}_{label}")
        return t[:, :], tag

    def packed_rot(self, pairs, c_src, s_src, sign, dst_ap, dst_pers,
                   A_ready=None):
        """Rotate k (y0, y1) pairs by the same z-angle in packed wide ops.

        pairs: list of (Y0_expr, Y1_expr) (ignored when A_ready given)
        c_src: (ap, scale) with cos == scale*ap;  s_src likewise for sin
        sign=+1: z0 = c y0 + s y1, z1 = -s y0 + c y1   (Rz^T)
        sign=-1: z0 = c y0 - s y1, z1 = +s y0 + c y1   (Rz)
        dst_ap: [P, 2k*F] region; slots (2i, 2i+1) <- (z0_i, z1_i)
        A_ready: optional (A_ap_region, [(a0, a1), ...]) when operands are
                 already adjacent raw planes with coefs.
        Returns [(z0_expr, z1_expr), ...] with pending 1.
        """
        k = len(pairs) if A_ready is None else len(A_ready[1])
        c_ap, c_sc = c_src
        s_ap, s_sc = s_src
        if A_ready is None:
            W = self.wscratch(2 * k)
            coefs = []
            consts = []
            for i, (Y0, Y1) in enumerate(pairs):
                a0, _ = self.mat(Y0, dst=W[:, (2 * i) * F:(2 * i + 1) * F])
                a1, _ = self.mat(Y1, dst=W[:, (2 * i + 1) * F:(2 * i + 2) * F])
                coefs.append((a0, a1))
                consts.append((Y0.const, Y1.const))
            A = W
        else:
            A, coefs, consts = A_ready
        B1 = self.wscratch(2 * k)
        B2 = self.wscratch(2 * k)
        for i, (a0, a1) in enumerate(coefs):
            s0 = slice((2 * i) * F, (2 * i + 1) * F)
            s1 = slice((2 * i + 1) * F, (2 * i + 2) * F)
            # B1: [a0*c, sign*a1*s]; B2: [-sign*a0*s, a1*c]
            self.nc.scalar.activation(B1[:, s0], c_ap, AF.Copy, bias=0.0,
                                      scale=float(a0 * c_sc))
            self.nc.scalar.activation(B1[:, s1], s_ap, AF.Copy, bias=0.0,
                                      scale=float(sign * a1 * s_sc))
            self.nc.scalar.activation(B2[:, s0], s_ap, AF.Copy, bias=0.0,
                                      scale=float(-sign * a0 * s_sc))
            self.nc.scalar.activation(B2[:, s1], c_ap, AF.Copy, bias=0.0,
                                      scale=float(a1 * c_sc))
            self.n_act += 4
        P1 = self.wscratch(2 * k)
        P2 = self.wscratch(2 * k)
        self.nc.vector.tensor_tensor(P1, A, B1, OP.mult)
        self.nc.vector.tensor_tensor(P2, A, B2, OP.mult)
        self.n_tt += 2
        # pairwise sums: z0_i = P1[2i] + P1[2i+1]; z1_i = P2[2i] + P2[2i+1]
        P1r = P1.rearrange("p (v c b) -> p v c b", c=2, b=F)
        P2r = P2.rearrange("p (v c b) -> p v c b", c=2, b=F)
        Dr = dst_ap.rearrange("p (v c b) -> p v c b", c=2, b=F)
        self.nc.vector.scalar_tensor_tensor(
            Dr[:, :, 0, :], P1r[:, :, 1, :], 1.0, P1r[:, :, 0, :],
            OP.mult, OP.add)
        self.nc.vector.scalar_tensor_tensor(
            Dr[:, :, 1, :], P2r[:, :, 1, :], 1.0, P2r[:, :, 0, :],
            OP.mult, OP.add)
        self.n_stt += 2
        out = []
        for i in range(k):
            z0_ap = dst_ap[:, (2 * i) * F:(2 * i + 1) * F]
            z1_ap = dst_ap[:, (2 * i + 1) * F:(2 * i + 2) * F]
            if dst_pers:
                self.pers_ids.add(id(z0_ap))
                self.pers_ids.add(id(z1_ap))
            k0, k1 = consts[i]
            # deferred additive consts of y0/y1 rotate into cos/sin terms:
            #   z0 += k0*cos + sign*k1*sin ; z1 += -sign*k0*sin + k1*cos
            t0 = [(1.0, z0_ap)]
            t1 = [(1.0, z1_ap)]
            if k0 != 0.0:
                t0.append((k0 * c_sc, c_ap))
                t1.append((-sign * k0 * s_sc, s_ap))
            if k1 != 0.0:
                t0.append((sign * k1 * s_sc, s_ap))
                t1.append((k1 * c_sc, c_ap))
            e0 = Expr(t0)
            e1 = Expr(t1)
            if len(t0) == 1:
                e0._mat = (1.0, z0_ap)
            if len(t1) == 1:
                e1._mat = (1.0, z1_ap)
            out.append((e0, e1))
        return out

    def free_expr_vec(self, vec):
        for e in vec:
            for _, ap in e.terms:
                ent = self.ap_tag.pop(id(ap), None)
                if ent is None:
                    continue
                self.pers_ids.discard(id(ap))
                if ent[0] == "p":
                    self.free_tags.append(ent[1])
                else:
                    self.free_wide.setdefault(ent[1], []).append(ent[2])

    def joint_boundary(self):
        self.max_joint_allocs = max(self.max_joint_allocs, self.joint_allocs)
        self.joint_allocs = 0

    # ---- expression ops ----
    def lin(self, *pairs, const=0.0):
        acc = {}
        aps = {}
        c_acc = float(const)
        for coef, e in pairs:
            if coef == 0.0 or e is None or e is ZERO and e.const == 0.0:
                if e is not None:
                    c_acc += coef * e.const
                continue
            c_acc += coef * e.const
            for tc, ap in e.terms:
                k = id(ap)
                acc[k] = acc.get(k, 0.0) + coef * tc
                aps[k] = ap
        terms = [(c, aps[k]) for k, c in acc.items() if c != 0.0]
        return Expr(terms, c_acc)

    def mat(self, e, dst=None):
        """Materialize sum-of-terms: e == coef*ap + e.const -> (coef, ap)."""
        assert e.terms, "cannot materialize empty expr"
        if e._mat is not None and dst is None:
            return e._mat
        terms = sorted(e.terms, key=lambda t: -abs(t[0]))
        if len(terms) == 1 and dst is None:
            e._mat = (terms[0][0], terms[0][1])
            return e._mat
        c0, x0 = terms[0]
        if len(terms) == 1:
            self.nc.scalar.activation(dst, x0, AF.Copy, bias=0.0, scale=1.0)
            self.n_copy += 1
            e._mat = (c0, dst)
            return e._mat
        t = dst if dst is not None else self.scratch()
        if not hasattr(self, "mat_hist"):
            self.mat_hist = {}
        self.mat_hist[len(terms)] = self.mat_hist.get(len(terms), 0) + 1
        c1, x1 = terms[1]
        eng = self.pick_engine(len(terms) - 1)
        eng.scalar_tensor_tensor(t, x1, c1 / c0, x0, OP.mult, OP.add)
        self.n_stt += 1
        for ck, xk in terms[2:]:
            eng.scalar_tensor_tensor(t, xk, ck / c0, t, OP.mult, OP.add)
            self.n_stt += 1
        e._mat = (c0, t)
        return e._mat

    def mul(self, x, y):
        if not _nonzero(x) or not _nonzero(y):
            return ZERO
        if not x.terms:  # pure const
            return Expr([(x.const * c, ap) for c, ap in y.terms],
                        x.const * y.const)
        if not y.terms:
            return Expr([(y.const * c, ap) for c, ap in x.terms],
                        x.const * y.const)
        cx, ax = self.mat(x)
        cy, ay = self.mat(y)
        prod = self.scratch()
        self.pick_engine(1).tensor_tensor(prod, ax, ay, OP.mult)
        self.n_tt += 1
        terms = [(cx * cy, prod)]
        if y.const != 0.0:
            terms.append((cx * y.const, ax))
        if x.const != 0.0:
            terms.append((cy * x.const, ay))
        return Expr(terms, x.const * y.const)

    def snap(self, e, label="", scratch_ok=False):
        """Materialize into a stable plane; returns single-term Expr."""
        if not e.terms:
            return e
        if len(e.terms) == 1 and e._mat is None \
                and id(e.terms[0][1]) in self.pers_ids and not scratch_ok:
            out = Expr(list(e.terms), e.const)
            out._mat = e.terms[0]
            return out
        if e._mat is not None:
            c, src = e._mat
            if id(src) in self.pers_ids or scratch_ok:
                out = Expr([(c, src)], e.const)
                out._mat = (c, src)
                return out
            dst = self.persistent(label)
            self.nc.scalar.activation(dst, src, AF.Copy, bias=0.0, scale=1.0)
            self.n_copy += 1
            out = Expr([(c, dst)], e.const)
            out._mat = (c, dst)
            return out
        dst = self.scratch() if scratch_ok else self.persistent(label)
        c, ap = self.mat(e, dst=dst)
        out = Expr([(c, ap)], e.const)
        out._mat = (c, ap)
        return out

    def snap_to(self, e, dst_ap):
        """Materialize into the given plane; returns single-term Expr."""
        assert e.terms
        c, ap = self.mat(e, dst=dst_ap)
        self.pers_ids.add(id(ap))
        out = Expr([(c, ap)], e.const)
        out._mat = (c, ap)
        return out

    def snap_vec(self, vec, label="", scratch_ok=False):
        return [self.snap(e, f"{label}{i}", scratch_ok) for i, e in enumerate(vec)]

    # ---- 3-vector helpers ----
    def vadd(self, *vecs):
        return [self.lin(*[(1.0, v[i]) for v in vecs]) for i in range(3)]

    def vsub(self, a, b):
        return [self.lin((1.0, a[i]), (-1.0, b[i])) for i in range(3)]

    def cross_const(self, t, X):
        return [
            self.lin((-t[2], X[1]), (t[1], X[2])),
            self.lin((t[2], X[0]), (-t[0], X[2])),
            self.lin((-t[1], X[0]), (t[0], X[1])),
        ]

    def cross_ee(self, A, B):
        return [
            self.lin((1.0, self.mul(A[1], B[2])), (-1.0, self.mul(A[2], B[1]))),
            self.lin((1.0, self.mul(A[2], B[0])), (-1.0, self.mul(A[0], B[2]))),
            self.lin((1.0, self.mul(A[0], B[1])), (-1.0, self.mul(A[1], B[0]))),
        ]

    def matvec_const(self, M, X):
        return [
            self.lin((M[i, 0], X[0]), (M[i, 1], X[1]), (M[i, 2], X[2]))
            for i in range(3)
        ]


def build_module(params):
    trans = np.asarray(params["trans"], np.float64)
    rot_fix = np.asarray(params["rot_fix"], np.float64)
    mass = np.asarray(params["mass"], np.float64)
    com = np.asarray(params["com"], np.float64)
    inertia = np.asarray(params["inertia"], np.float64)
    damping = np.asarray(params["damping"], np.float64)

    nc = bacc.Bacc("TRN2", target_bir_lowering=False, debug=False,
                   num_devices=N_CORES)
    q_d = nc.dram_tensor("q", (BC, ND), DT, kind="ExternalInput")
    qd_d = nc.dram_tensor("qd", (BC, ND), DT, kind="ExternalInput")
    qdd_d = nc.dram_tensor("qdd_des", (BC, ND), DT, kind="ExternalInput")
    tq_d = nc.dram_tensor("torque", (BC, ND), DT, kind="ExternalOutput")

    with tile_mod.TileContext(nc) as tc:
        with tc.tile_pool(name="main", bufs=1) as pool, \
             tc.tile_pool(name="io", bufs=1) as io_pool:
            b = Builder(nc, pool, ring_size=64,
                        pool_frac=float(os.environ.get("K_POOL_FRAC", "0")))

            q_t = io_pool.tile([P, F * ND], DT, tag="q", name="q_sb")
            qd_t = io_pool.tile([P, F * ND], DT, tag="qd", name="qd_sb")
            qdd_t = io_pool.tile([P, F * ND], DT, tag="qdd", name="qdd_sb")
            out_t = io_pool.tile([P, F * ND], DT, tag="out", name="out_sb")
            dram_view = lambda t: t.ap().rearrange("(p f) d -> p (f d)", p=P)
            nc.sync.dma_start(q_t[:, :], dram_view(q_d))
            nc.sync.dma_start(qd_t[:, :], dram_view(qd_d))
            nc.sync.dma_start(qdd_t[:, :], dram_view(qdd_d))

            q3 = q_t[:, :].rearrange("p (f d) -> p f d", d=ND)
            qd3 = qd_t[:, :].rearrange("p (f d) -> p f d", d=ND)
            qdd3 = qdd_t[:, :].rearrange("p (f d) -> p f d", d=ND)
            out3 = out_t[:, :].rearrange("p (f d) -> p f d", d=ND)

            def as_pers_expr(ap):
                b.pers_ids.add(id(ap))
                e = Expr([(1.0, ap)])
                e._mat = (1.0, ap)
                return e

            def const_col(tag, val):
                t = io_pool.tile([P, 1], DT, tag=tag, name=tag)
                nc.vector.memset(t[:, :], float(val))
                return t

            zero_t = const_col("zconst", 0.0)

            # ACT Sin has no range reduction (accurate only on [-pi, pi]).
            # q ~ N(0,1) so |q| <= ~5.5: one conditional wrap of 2*pi covers
            # sin(q) and cos(q) = sin((q + pi/2) wrapped).
            PI = float(np.pi)
            TWO_PI = float(2 * np.pi)
            trig = {}
            trig_raw = {}
            _kp = os.environ.get("K_PACK", "1")
            use_pack_fwd = _kp in ("1", "fwd")
            use_pack_bwd = _kp in ("1", "bwd")
            for j in range(ND - 1):  # joints 0..5
                # ACT Sin is only accurate on [-pi, pi] (no range reduction)
                # and q ~ N(0,1) exceeds pi. Half-angle identities avoid any
                # wrapping: |q/2| <= ~2.9 < pi and |q/4| < pi/2 always.
                #   cos q = 1 - 2 sin^2(q/2)
                #   sin q = 2 sin(q/2) cos(q/2),  cos(q/2) = 1 - 2 sin^2(q/4)
                # Everything runs on the otherwise-idle ACT engine except one
                # DVE product for sin.
                qj = q3[:, :, j]
                c_ap = b.persistent(f"c{j}")
                s_ap = b.persistent(f"s{j}")
                sh = b.scratch(DT)   # sin(q/2)
                nc.scalar.activation(sh, qj, AF.Sin, bias=zero_t[:, :],
                                     scale=0.5)
                sq = b.scratch(DT)   # sin(q/4)
                nc.scalar.activation(sq, qj, AF.Sin, bias=zero_t[:, :],
                                     scale=0.25)
                sq2 = b.scratch(DT)  # sin^2(q/4)
                nc.scalar.activation(sq2, sq, AF.Square)
                ch = b.scratch(DT)   # cos(q/2)
                nc.scalar.activation(ch, sq2, AF.Copy, bias=1.0, scale=-2.0)
                sh2 = b.scratch(DT)  # sin^2(q/2)
                nc.scalar.activation(sh2, sh, AF.Square)
                nc.scalar.activation(c_ap, sh2, AF.Copy, bias=1.0, scale=-2.0)
                nc.vector.tensor_tensor(s_ap, sh, ch, OP.mult)  # sin q / 2
                b.n_tt += 1
                b.n_act += 6
                se = Expr([(2.0, s_ap)])
                se._mat = (2.0, s_ap)
                trig[j] = (as_pers_expr(c_ap), se)
                trig_raw[j] = (c_ap, 1.0, s_ap, 2.0)
            b.joint_boundary()

            qd_pl = []
            for j in range(ND):
                d_ap = b.persistent(f"qd{j}")
                nc.scalar.activation(d_ap, qd3[:, :, j], AF.Copy,
                                     bias=0.0, scale=1.0)
                b.n_act += 1
                qd_pl.append(as_pers_expr(d_ap))

            qdd_cache = {}

            def qdd_expr(j):
                if DT_C != DT:
                    if j not in qdd_cache:
                        d_ap = b.persistent(f"qdd{j}")
                        nc.scalar.activation(d_ap, qdd3[:, :, j], AF.Copy,
                                             bias=0.0, scale=1.0)
                        b.n_act += 1
                        qdd_cache[j] = as_pers_expr(d_ap)
                    return qdd_cache[j]
                ap = qdd3[:, :, j]
                e = Expr([(1.0, ap)])
                e._mat = (1.0, ap)
                b.pers_ids.add(id(ap))
                return e

            def rot_inv(j, X):
                """Rz(q_j)^T @ (rot_fix_j^T @ X)"""
                if not any(_nonzero(e) for e in X):
                    return [ZERO, ZERO, ZERO]
                Fm = rot_fix[j].T
                Y = b.matvec_const(Fm, X)
                c, s = trig[j]
                z0 = b.lin((1.0, b.mul(c, Y[0])), (1.0, b.mul(s, Y[1])))
                z1 = b.lin((-1.0, b.mul(s, Y[0])), (1.0, b.mul(c, Y[1])))
                return [z0, z1, Y[2]]

            def write_out(j, e):
                dst = out3[:, :, j]
                if not e.terms:
                    b.nc.vector.memset(dst, float(e.const))
                    return
                c, ap = b.mat(e)
                nc.scalar.activation(dst, ap, AF.Copy, bias=float(e.const),
                                     scale=float(c))
                b.n_act += 1

            # ---------------- forward (joints 0..5) ----------------
            w_p = [ZERO, ZERO, ZERO]
            v_p = [ZERO, ZERO, ZERO]
            a_p = [ZERO, ZERO, ZERO]
            la_p = [ZERO, ZERO, Expr(const=GRAV)]
            states = []
            for j in range(ND - 1):
                t_j = trans[j]
                Uv = b.vsub(v_p, b.cross_const(t_j, w_p))
                Ua = b.vsub(la_p, b.cross_const(t_j, a_p))
                Uv = [b.snap(e, f"Uv{j}", scratch_ok=True)
                      if len(e.terms) > 1 else e for e in Uv]
                Ua = [b.snap(e, f"Ua{j}", scratch_ok=True)
                      if len(e.terms) > 1 else e for e in Ua]
                packed_done = False
                _kpj = int(os.environ.get("K_PACK_J", "-1"))
                if use_pack_fwd and j > 0 and (_kpj < 0 or j == _kpj):
                    Fm = rot_fix[j].T

                    def _rows(X):
                        return [b.lin((Fm[i, 0], X[0]), (Fm[i, 1], X[1]),
                                      (Fm[i, 2], X[2])) for i in range(3)]

                    Yw, Yv, Ya, Yu = _rows(w_p), _rows(Uv), _rows(a_p), _rows(Ua)
                    heads = [Yw[0], Yw[1], Yv[0], Yv[1],
                             Ya[0], Ya[1], Yu[0], Yu[1]]
                    if all(e.terms for e in heads):
                        c_ap, c_sc, s_ap, s_sc = trig_raw[j]
                        wv_tile, wv_tag = b.persistent_wide(4, f"wv{j}")
                        (z_w, z_v) = b.packed_rot(
                            [(Yw[0], Yw[1]), (Yv[0], Yv[1])],
                            (c_ap, c_sc), (s_ap, s_sc), +1, wv_tile, True)
                        b.ap_tag[id(z_w[0].terms[0][1])] = ("w", 4, wv_tag)
                        ar_tile = b.wscratch(4)
                        (z_a, z_u) = b.packed_rot(
                            [(Ya[0], Ya[1]), (Yu[0], Yu[1])],
                            (c_ap, c_sc), (s_ap, s_sc), +1, ar_tile, False)
                        Rw = [z_w[0], z_w[1], Yw[2]]
                        Rv = [z_v[0], z_v[1], Yv[2]]
                        Ra = [z_a[0], z_a[1], Ya[2]]
                        Rla = [z_u[0], z_u[1], Yu[2]]
                        packed_done = True
                if not packed_done:
                    Rw = rot_inv(j, w_p)
                    Rv = rot_inv(j, Uv)
                    Ra = rot_inv(j, a_p)
                    Rla = rot_inv(j, Ua)
                qdj = qd_pl[j]
                qddj = qdd_expr(j)
                w = [Rw[0], Rw[1], b.lin((1.0, Rw[2]), (1.0, qdj))]
                w = b.snap_vec(w, f"w{j}_")
                v = b.snap_vec(Rv, f"v{j}_")
                dw = [
                    b.lin((1.0, Ra[0]), (1.0, b.mul(w[1], qdj))),
                    b.lin((1.0, Ra[1]), (-1.0, b.mul(w[0], qdj))),
                    b.lin((1.0, Ra[2]), (1.0, qddj)),
                ]
                dv = [
                    b.lin((1.0, Rla[0]), (1.0, b.mul(v[1], qdj))),
                    b.lin((1.0, Rla[1]), (-1.0, b.mul(v[0], qdj))),
                    Rla[2],
                ]
                dw = b.snap_vec(dw, f"dw{j}_")
                dv = b.snap_vec(dv, f"dv{j}_")
                states.append((w, v, dw, dv))
                w_p, v_p, a_p, la_p = w, v, dw, dv
                b.joint_boundary()

            # ---------------- backward (j = 5..0) ----------------
            write_out(ND - 1, b.lin((damping[ND - 1], qd_pl[ND - 1])))

            lin_f = [ZERO, ZERO, ZERO]
            ang_f = [ZERO, ZERO, ZERO]
            bw_pack = None
            for j in range(ND - 2, -1, -1):
                # free state planes of joint j+1 once consumed (previous iter)
                if j != ND - 2:
                    for vec in states[j + 1]:
                        b.free_expr_vec(vec)

                have_child = any(_nonzero(e) for e in lin_f + ang_f)
                if have_child and use_pack_bwd and bw_pack is not None:
                    A_ap, coefs, kconsts, prev_tag = bw_pack
                    c_ap, c_sc, s_ap, s_sc = trig_raw[j + 1]
                    Rf = rot_fix[j + 1]
                    rz = b.wscratch(4)
                    (zlf, zaf) = b.packed_rot(
                        None, (c_ap, c_sc), (s_ap, s_sc), -1, rz, False,
                        A_ready=(A_ap, coefs, kconsts))
                    b.free_wide.setdefault(4, []).append(prev_tag)
                    RzLf = [zlf[0], zlf[1], lin_f[2]]
                    RzAf = [zaf[0], zaf[1], ang_f[2]]
                    Rc_lf = b.matvec_const(Rf, RzLf)
                    Rc_lf = [b.snap(e, f"rclf{j}", scratch_ok=True)
                             if len(e.terms) > 2 else e for e in Rc_lf]
                    Rc_af = b.matvec_const(Rf, RzAf)
                    child_ang = b.vadd(b.cross_const(trans[j + 1], Rc_lf), Rc_af)
                    child_lin = Rc_lf
                elif have_child:
                    cs, ss = trig[j + 1]
                    Rf = rot_fix[j + 1]
                    # xy-rotation outputs fan out 3x through the rot_fix
                    # matvec: snapping them saves (t-1)(f-1) chain slots.
                    RzLf = [
                        b.snap(b.lin((1.0, b.mul(cs, lin_f[0])),
                                     (-1.0, b.mul(ss, lin_f[1]))),
                               f"rzlf{j}0", scratch_ok=True),
                        b.snap(b.lin((1.0, b.mul(ss, lin_f[0])),
                                     (1.0, b.mul(cs, lin_f[1]))),
                               f"rzlf{j}1", scratch_ok=True),
                        lin_f[2],
                    ]
                    Rc_lf = b.matvec_const(Rf, RzLf)
                    Rc_lf = [b.snap(e, f"rclf{j}", scratch_ok=True)
                             if len(e.terms) > 2 else e for e in Rc_lf]
                    RzAf = [
                        b.snap(b.lin((1.0, b.mul(cs, ang_f[0])),
                                     (-1.0, b.mul(ss, ang_f[1]))),
                               f"rzaf{j}0", scratch_ok=True),
                        b.snap(b.lin((1.0, b.mul(ss, ang_f[0])),
                                     (1.0, b.mul(cs, ang_f[1]))),
                               f"rzaf{j}1", scratch_ok=True),
                        ang_f[2],
                    ]
                    Rc_af = b.matvec_const(Rf, RzAf)
                    child_ang = b.vadd(b.cross_const(trans[j + 1], Rc_lf), Rc_af)
                    child_lin = Rc_lf
                else:
                    child_ang = [ZERO, ZERO, ZERO]
                    child_lin = [ZERO, ZERO, ZERO]

                w, v, dw, dv = states[j]
                m = float(mass[j])
                mc = m * com[j]
                cxm = np.array([
                    [0.0, -com[j][2], com[j][1]],
                    [com[j][2], 0.0, -com[j][0]],
                    [-com[j][1], com[j][0], 0.0],
                ])
                Isp = inertia[j] + m * (cxm @ cxm.T)

                IcA_l = b.vsub([b.lin((m, dv[i])) for i in range(3)],
                               b.cross_const(mc, dw))
                IcA_a = b.vadd(b.matvec_const(Isp, dw), b.cross_const(mc, dv))
                IcV_l = b.vsub([b.lin((m, v[i])) for i in range(3)],
                               b.cross_const(mc, w))
                IcV_a = b.vadd(b.matvec_const(Isp, w), b.cross_const(mc, v))
                tmp_a = b.vadd(b.cross_ee(w, IcV_a), b.cross_ee(v, IcV_l))
                tmp_l = b.cross_ee(w, IcV_l)
                lf_new = b.vadd(IcA_l, tmp_l, child_lin)
                af_new = b.vadd(IcA_a, tmp_a, child_ang)
                if use_pack_bwd and j > 0:
                    bw_tile, bw_tag = b.persistent_wide(4, f"bw{j}")
                    lf0 = b.snap_to(lf_new[0], bw_tile[:, 0:F])
                    lf1 = b.snap_to(lf_new[1], bw_tile[:, F:2 * F])
                    af0 = b.snap_to(af_new[0], bw_tile[:, 2 * F:3 * F])
                    af1 = b.snap_to(af_new[1], bw_tile[:, 3 * F:4 * F])
                    lf2 = b.snap(lf_new[2], f"lf{j}2")
                    af2 = b.snap(af_new[2], f"af{j}2")
                    lin_f = [lf0, lf1, lf2]
                    ang_f = [af0, af1, af2]
                    bw_pack = (bw_tile, [(lf0._mat[0], lf1._mat[0]),
                                         (af0._mat[0], af1._mat[0])],
                               [(lf0.const, lf1.const),
                                (af0.const, af1.const)], bw_tag)
                else:
                    lin_f = b.snap_vec(lf_new, f"lf{j}_")
                    ang_f = b.snap_vec(af_new, f"af{j}_")
                    bw_pack = None
                write_out(j, b.lin((1.0, ang_f[2]), (damping[j], qd_pl[j])))
                b.joint_boundary()

            nc.sync.dma_start(dram_view(tq_d), out_t[:, :])

            stats = dict(stt=b.n_stt, tt=b.n_tt, act=b.n_act, copy=b.n_copy,
                         pers=b.pers_idx, max_joint_allocs=b.max_joint_allocs,
                         ring=b.ring_size,
                         mat_hist=dict(sorted(getattr(b, "mat_hist", {}).items())))

    nc.compile()
    return nc, stats


_CACHE = {}


def _get_module(params):
    import hashlib
    key = b"".join(np.ascontiguousarray(np.asarray(params[k], np.float32)).tobytes()
                   for k in ("trans", "rot_fix", "mass", "com", "inertia",
                             "damping"))
    h = hashlib.sha1(key).hexdigest()
    if h not in _CACHE:
        _CACHE[h] = build_module(params)
    return _CACHE[h]


def run(q, qd, qdd_des, trans, rot_fix, mass, com, inertia, damping,
        trace=False):
    q = np.asarray(q)
    qd = np.asarray(qd)
    qdd_des = np.asarray(qdd_des)
    assert q.shape == (B_TOTAL, ND), f"unexpected q shape {q.shape}"
    assert qd.shape == (B_TOTAL, ND) and qdd_des.shape == (B_TOTAL, ND)
    params = dict(trans=trans, rot_fix=rot_fix, mass=mass, com=com,
                  inertia=inertia, damping=damping)
    nc, stats = _get_module(params)
    in_maps = []
    for c in range(N_CORES):
        sl = slice(c * BC, (c + 1) * BC)
        in_maps.append({
            "q": np.ascontiguousarray(q[sl], np.float32),
            "qd": np.ascontiguousarray(qd[sl], np.float32),
            "qdd_des": np.ascontiguousarray(qdd_des[sl], np.float32),
        })
    res = bass_utils.run_bass_kernel_spmd(
        nc, in_maps, core_ids=list(range(N_CORES)), trace=trace)
    out = np.concatenate([res.results[c]["torque"] for c in range(N_CORES)],
                         axis=0)
    return out.astype(np.float32), res, stats


def kernel(q, qd, qdd_des, trans, rot_fix, mass, com, inertia, damping):
    out, _, _ = run(q, qd, qdd_des, trans, rot_fix, mass, com, inertia,
                    damping, trace=False)
    return out
